# revision 41
# baseline (speedup 1.0000x reference)
"""Trainium2 Bass kernel for the butterfly-CNN problem (nn_CNNLayer_30296699306356).

Network (see problem reference): input conv (k=2,s=2, 1->8 ch) + 10 butterfly
conv levels (k=2,s=2, channels double each level, relu, zero biases) + a
per-block dense matmul (1024 blocks of [8,2]) at the end.

Strategy (memory-regime; weights are ~358 MB fp32 dominated by levels 8-10):
  - Levels in..7 are replicated on all 8 cores (small weights, ~11 MB).
    Levels in..4 use an "im2col-packed" layout: activations are stored as
    [128 partitions = (wsub, ch), wHi, b] so every matmul is a full 128-wide
    contraction with a single block-structured 128x128 stationary weight.
  - Levels 8, 9, 10 shard the OUTPUT channels across the 8 cores (1/8 of the
    weight traffic per core). Activations are re-assembled with an 8-core
    AllGather after levels 8 and 9. Level 10's output channels align exactly
    with the fea_dense block shard, so no gather is needed after level 10.
  - Level 10 runs "orientation B" (activations stationary, weights moving) so
    the 32 MB/core of f10 streams through the PE at N=512 per matmul.
  - Final block einsum is done on the Vector engine (mult + grouped reduce).

kernel(**inputs) takes the FULL unsharded inputs and returns the FULL output.
"""

import ml_dtypes
import numpy as np

NCORES = 8
B = 16
P = 128
C = 8
NLVL = 10
BF16 = ml_dtypes.bfloat16

_CACHE = {}


# ---------------------------------------------------------------- host prep

def _host_prep(inputs):
    """Build the per-core input maps (numpy only)."""
    ind = np.ascontiguousarray(np.asarray(inputs["in_data"], np.float32))
    f = {l: np.asarray(inputs[f"f{l}"], np.float32) for l in range(1, NLVL + 1)}
    f0 = np.asarray(inputs["in_filter"], np.float32)     # [2, 1, 8]
    fd = np.asarray(inputs["fea_dense"], np.float32)     # [1024, 8, 2]

    shared = {}
    # r0 [32, 64, 16]: r0[row, wHi, b] = in[b, wHi*32 + row]
    shared["r0"] = np.ascontiguousarray(
        ind[:, :, 0].reshape(B, 64, 32).transpose(2, 1, 0))

    # w0 [32, 128]: rows (2*wsub + k), cols (wsub*8 + co)
    w0 = np.zeros((32, 128), np.float32)
    for wsub in range(16):
        for k in range(2):
            w0[2 * wsub + k, wsub * 8:wsub * 8 + 8] = f0[k, 0, :]
    shared["w0"] = w0

    # packed levels 1..4 stacked: wpk [4, 128, 128]
    wpk = np.zeros((4, 128, 128), np.float32)
    for lvl in range(1, 5):
        cin = 2 ** (lvl - 1) * C
        cout = 2 ** lvl * C
        s_out = (128 // cin) // 2
        for wso in range(s_out):
            for k in range(2):
                wsi = 2 * wso + k
                wpk[lvl - 1, wsi * cin:(wsi + 1) * cin,
                    wso * cout:(wso + 1) * cout] = f[lvl][k]
    shared["wpk"] = wpk

    # w5/w6/w7 mega-packed [128, 10752] bf16 (kt-major per level), one DMA
    w5v = f[5].astype(BF16).reshape(2, 1, 128, 256)
    w6v = f[6].astype(BF16).reshape(2, 2, 128, 512)
    w7v = f[7].astype(BF16).reshape(2, 4, 128, 1024)
    shared["wmid"] = np.ascontiguousarray(np.concatenate([
        w5v.transpose(2, 0, 1, 3).reshape(128, 512),
        w6v.transpose(2, 0, 1, 3).reshape(128, 2048),
        w7v.transpose(2, 0, 1, 3).reshape(128, 8192)], axis=1))

    # f8/f9/f10 output-channel shards, packed into 4-ci-tile chunks:
    # [nchunks, 128, 4, cout_shard]; chunk m = k*(cin//512) + q, cit = q*4+j
    def shard_pack(fl, cin):
        sh = fl.shape[-1] // NCORES
        out = []
        flb = fl.astype(BF16)
        for r in range(NCORES):
            blk = flb[:, :, r * sh:(r + 1) * sh]
            v = blk.reshape(2, cin // 512, 4, 128, sh).transpose(0, 1, 3, 2, 4)
            out.append(np.ascontiguousarray(
                v.reshape(2 * (cin // 512), 128, 4, sh)))
        return out

    # f8 is REPLICATED (cheaper than the extra AllGather): co-major chunks
    # [4, 128, kt=16, co=512], kt = k*8 + cit
    f8b = f[8].astype(BF16)
    w8full = np.stack([
        np.ascontiguousarray(
            f8b[:, :, c * 512:(c + 1) * 512]
            .reshape(2, 8, 128, 512).transpose(2, 0, 1, 3).reshape(128, 16, 512))
        for c in range(4)])
    shared["w8"] = w8full

    w9s = shard_pack(f[9], 2048)    # 8 chunks of [128, 4, 512]

    # f10 is K-SHARDED (input channels): core r holds rows [512r, 512r+512)
    # for ALL 8192 outputs, so level 10 needs only the core's own x9loc and
    # the cross-core sum happens in one ReduceScatter at the very end.
    # co-major chunks: [16, 128, kt=8, 512], kt = k*4 + t_loc
    f10b = f[10].astype(BF16)
    w10s = []
    for r in range(NCORES):
        blk = f10b[:, 512 * r:512 * (r + 1), :]          # [2, 512, 8192]
        v = blk.reshape(2, 4, 128, 16, 512).transpose(3, 2, 0, 1, 4)
        w10s.append(np.ascontiguousarray(v.reshape(16, 128, 8, 512)))

    # fea_dense shard, per-o flattened, tiled over the 16 batch partitions
    fds = []
    for r in range(NCORES):
        blk = fd[r * 128:(r + 1) * 128]                    # [128, 8, 2]
        flat = blk.transpose(2, 0, 1).reshape(2, 1024)     # [o, 1024]
        fds.append(np.ascontiguousarray(
            np.broadcast_to(flat[None], (B, 2, 1024))))

    in_maps = []
    for r in range(NCORES):
        m = dict(shared)
        m["w9"] = w9s[r]
        m["w10"] = w10s[r]
        m["fdt"] = fds[r]
        in_maps.append(m)
    return in_maps


# ---------------------------------------------------------------- bass build

def _build():
    import concourse.bass as bass
    import concourse.mybir as mybir
    import concourse.tile as tile
    from concourse import bacc

    f32 = mybir.dt.float32
    bf16 = mybir.dt.bfloat16
    RELU = mybir.ActivationFunctionType.Relu

    nc = bacc.Bacc("TRN2", target_bir_lowering=False, debug=False,
                   num_devices=NCORES)

    def inp(name, shape, dt=f32):
        return nc.dram_tensor(name, shape, dt, kind="ExternalInput").ap()

    r0 = inp("r0", [32, 64, 16])
    w0 = inp("w0", [32, 128])
    wpk = inp("wpk", [4, 128, 128])
    wmid = inp("wmid", [128, 10752], bf16)
    w8 = inp("w8", [4, 128, 16, 512], bf16)
    w9 = inp("w9", [8, 128, 4, 512], bf16)
    w10 = inp("w10", [16, 128, 8, 512], bf16)
    fdt = inp("fdt", [B, 2, 1024])
    out = nc.dram_tensor("out", [B, 128, 2], f32, kind="ExternalOutput").ap()

    with tile.TileContext(nc) as tc:
        with (
            tc.tile_pool(name="const", bufs=1) as constp,
            tc.tile_pool(name="actp", bufs=2) as actp,
            tc.tile_pool(name="bigp", bufs=1) as bigp,
            tc.tile_pool(name="w7p", bufs=1) as w7p,
            tc.tile_pool(name="w8p", bufs=2) as w8p,
            tc.tile_pool(name="w9p", bufs=4) as w9p,
            tc.tile_pool(name="w10p", bufs=8) as w10p,
            tc.tile_pool(name="psA", bufs=2, space="PSUM") as psA,
            tc.tile_pool(name="psB", bufs=4, space="PSUM") as psB,
            tc.tile_pool(name="psC", bufs=2, space="PSUM") as psC,
            tc.tile_pool(name="dramp", bufs=1, space="DRAM") as dramp,
        ):
            # ---- resident loads
            r0sb = constp.tile([32, 64, 16], f32, name="r0sb")
            nc.sync.dma_start(r0sb[:], r0)
            w0sb = constp.tile([32, 128], f32, name="w0sb")
            nc.sync.dma_start(w0sb[:], w0)
            wpksb = constp.tile([128, 4, 128], f32, name="wpksb")
            nc.sync.dma_start(wpksb[:], wpk.rearrange("l p c -> p l c"))
            wmidsb = w7p.tile([128, 10752], bf16, name="wmidsb")
            nc.sync.dma_start(wmidsb[:], wmid)
            w5sb = wmidsb[:, 0:512].rearrange("p (t c) -> p t c", c=256)
            w6sb = wmidsb[:, 512:2560].rearrange("p (t c) -> p t c", c=512)
            w7sb = wmidsb[:, 2560:10752].rearrange("p (t c) -> p t c", c=1024)
            fdsb = constp.tile([B, 2, 1024], f32, name="fdsb")
            nc.sync.dma_start(fdsb[:], fdt)

            # ---- input conv + packed levels 1..4 (all [128, 64, 16])
            xprev = None
            for lvl in range(5):
                # x4 feeds the bf16 level-5 matmul, so cast at the relu
                xn = actp.tile([128, 64, 16], bf16 if lvl == 4 else f32,
                               name=f"x{lvl}", tag="xl")
                for ch in range(2):
                    ps = psA.tile([128, 32, 16], f32, name="psA", tag="psA")
                    if lvl == 0:
                        nc.tensor.matmul(
                            ps[:], w0sb[:], r0sb[:, ch * 32:(ch + 1) * 32, :],
                            start=True, stop=True)
                    else:
                        nc.tensor.matmul(
                            ps[:], wpksb[:, lvl - 1, :],
                            xprev[:, ch * 32:(ch + 1) * 32, :],
                            start=True, stop=True)
                    nc.scalar.activation(
                        xn[:, ch * 32:(ch + 1) * 32, :], ps[:], RELU)
                xprev = xn

            # ---- standard levels (orientation A, weights stationary)
            def std_level(xin, wsb, cin_t, cout_t, w_out, name, out_tile=None):
                # xin [128, cin_t, 2*w_out, 16]; wsb [128, 2*cin_t, co] with
                # kt = k*cin_t + cit; returns [128, cout_t, w_out, 16]
                if out_tile is None:
                    xn = actp.tile([128, cout_t, w_out, 16], bf16,
                                   name=name, tag="xl")
                else:
                    xn = out_tile
                for ct in range(cout_t):
                    ps = psA.tile([128, w_out, 16], f32, name="psA", tag="psA")
                    for cit in range(cin_t):
                        rhs2 = xin[:, cit].rearrange(
                            "p (w two) b -> p two w b", two=2)
                        for k in range(2):
                            nc.tensor.matmul(
                                ps[:],
                                wsb[:, k * cin_t + cit,
                                    ct * 128:(ct + 1) * 128],
                                rhs2[:, k],
                                start=(cit == 0 and k == 0),
                                stop=(cit == cin_t - 1 and k == 1))
                    nc.scalar.activation(xn[:, ct], ps[:], RELU)
                return xn

            x5 = std_level(xprev[:, None], w5sb, 1, 2, 32, "x5")
            x6 = std_level(x5, w6sb, 2, 4, 16, "x6")
            x7 = std_level(x6, w7sb, 4, 8, 8, "x7")

            # ---- level 8 REPLICATED (full 2048 cout), co-major weight stream
            x8sb = bigp.tile([128, 16, 4, 16], bf16, name="x8sb")
            for c in range(4):
                w8c = w8p.tile([128, 16, 512], bf16, name="w8c", tag="w8c")
                nc.sync.dma_start(w8c[:], w8[c])
                for ctl in range(4):
                    ps = psA.tile([128, 4, 16], f32, name="psA", tag="psA")
                    for cit in range(8):
                        rhs2 = x7[:, cit].rearrange(
                            "p (w two) b -> p two w b", two=2)
                        for k in range(2):
                            nc.tensor.matmul(
                                ps[:],
                                w8c[:, k * 8 + cit, ctl * 128:(ctl + 1) * 128],
                                rhs2[:, k],
                                start=(cit == 0 and k == 0),
                                stop=(cit == 7 and k == 1))
                    nc.scalar.activation(x8sb[:, c * 4 + ctl], ps[:], RELU)

            # ---- level 9 (512-ch shard, streamed weights, 4 accumulators)
            ps9 = [psB.tile([128, 2, 16], f32, name=f"ps9_{ct}", tag="psB")
                   for ct in range(4)]
            for m in range(8):
                w9c = w9p.tile([128, 4, 512], bf16, name="w9c", tag="w9c")
                nc.sync.dma_start(w9c[:], w9[m])
                k, q = divmod(m, 4)
                for j in range(4):
                    cit = q * 4 + j
                    rhs = x8sb[:, cit].rearrange(
                        "p (w two) b -> p two w b", two=2)[:, k]
                    for ct in range(4):
                        nc.tensor.matmul(
                            ps9[ct][:],
                            w9c[:, j, ct * 128:(ct + 1) * 128],
                            rhs,
                            start=(m == 0 and j == 0),
                            stop=(m == 7 and j == 3))
            x9loc = bigp.tile([128, 4, 2, 16], bf16, name="x9loc")
            for ct in range(4):
                nc.scalar.activation(x9loc[:, ct], ps9[ct][:], RELU)

            # ---- level 10, K-sharded: partial sums over the core's own 512
            # x9 channels for ALL 8192 outputs (acts stationary, co-major
            # weight stream), partials kept in bf16 for the ReduceScatter
            x10p = bigp.tile([B, 16, 512], bf16, name="x10p")
            for g in range(16):
                w10c = w10p.tile([128, 8, 512], bf16, name="w10c", tag="w10c")
                nc.sync.dma_start(w10c[:], w10[g])
                ps = psC.tile([B, 512], f32, name="psC", tag="psC")
                for kt in range(8):
                    k, t = divmod(kt, 4)
                    nc.tensor.matmul(
                        ps[:], x9loc[:, t, k, :], w10c[:, kt, :],
                        start=(kt == 0), stop=(kt == 7))
                nc.vector.tensor_copy(x10p[:, g, :], ps[:])

            # ---- ReduceScatter the partials: core r receives the summed
            # outputs for blocks [128r, 128r+128) = its fea_dense shard
            rs_in = dramp.tile([NCORES, B, 1024], bf16, name="rs_in")
            rs_out = dramp.tile([1, B, 1024], bf16, name="rs_out")
            for rp in range(NCORES):
                nc.sync.dma_start(
                    rs_in[rp],
                    x10p[:, 2 * rp:2 * rp + 2, :].rearrange("p g c -> p (g c)"))
            nc.gpsimd.collective_compute(
                "ReduceScatter", mybir.AluOpType.add,
                replica_groups=[list(range(NCORES))],
                ins=[rs_in.opt()], outs=[rs_out.opt()])
            x10b = bigp.tile([B, 1024], bf16, name="x10b")
            nc.sync.dma_start(x10b[:], rs_out[0])
            x10 = bigp.tile([B, 1024], f32, name="x10")
            nc.scalar.activation(x10[:], x10b[:], RELU)

            # ---- final per-block einsum on the vector engine
            osb = bigp.tile([B, 128, 2], f32, name="osb")
            for o in range(2):
                prod = bigp.tile([B, 1024], f32, name=f"prod{o}")
                nc.vector.tensor_tensor(
                    prod[:], x10[:], fdsb[:, o, :], mybir.AluOpType.mult)
                nc.vector.tensor_reduce(
                    osb[:, :, o],
                    prod.rearrange("p (k c) -> p k c", c=8),
                    mybir.AxisListType.X, mybir.AluOpType.add)
            nc.sync.dma_start(out, osb[:])

    nc.compile()
    return nc


# ------------------------------------------------------------------- kernel

def kernel(**inputs):
    from concourse.bass_utils import run_bass_kernel_spmd

    in_maps = _host_prep(inputs)
    if "nc" not in _CACHE:
        _CACHE["nc"] = _build()
    nc = _CACHE["nc"]
    res = run_bass_kernel_spmd(nc, in_maps, core_ids=list(range(NCORES)))
    parts = [res.results[r]["out"] for r in range(NCORES)]  # each [16, 128, 2]
    full = np.concatenate(parts, axis=1)                    # [16, 1024, 2]
    return np.ascontiguousarray(full.reshape(B, 2048, 1).astype(np.float32))


# revision 47
# speedup vs baseline: 1.0272x; 1.0272x over previous
"""Trainium2 Bass kernel for the butterfly-CNN problem (nn_CNNLayer_30296699306356).

Network (see problem reference): input conv (k=2,s=2, 1->8 ch) + 10 butterfly
conv levels (k=2,s=2, channels double each level, relu, zero biases) + a
per-block dense matmul (1024 blocks of [8,2]) at the end.

Strategy (memory-regime; weights are ~358 MB fp32 dominated by levels 8-10):
  - Levels 5..10 run in bf16 (weights + activations, fp32 PSUM accumulation):
    halves the HBM weight traffic and makes matmuls single-pass on the PE
    (fp32 matmuls lower to two LO/HI passes on trn2). Levels in..4 stay fp32
    (earliest levels compound quantization error the most). Measured rel err
    vs the fp32 reference: ~5e-3.
  - Levels in..8 are replicated on all 8 cores. Levels in..4 use an
    "im2col-packed" layout: activations are stored as [128 partitions =
    (wsub, ch), wHi, b] so every matmul is a full 128-wide contraction with a
    single block-structured 128x128 stationary weight. Level 8 is replicated
    rather than sharded because an extra 7 MB bf16 of streamed weights is
    cheaper than a second 8-core collective (~40 us end-to-end observed).
  - Levels 9 and 10 shard the OUTPUT channels across the 8 cores (1/8 of the
    dominant weight traffic per core). One 8-core AllGather reassembles x9.
    Level 10's output channels align exactly with the fea_dense block shard,
    so no gather is needed after level 10.
  - Level 10 runs "orientation B" (activations stationary, weights moving) so
    its 16 MB/core bf16 stream feeds the PE at N=512 per matmul; deep tile
    pools let the stream prefetch through the AllGather latency window.
  - Final block einsum is done on the Vector engine (mult + grouped reduce).

kernel(**inputs) takes the FULL unsharded inputs and returns the FULL output.
"""

import ml_dtypes
import numpy as np

NCORES = 8
B = 16
P = 128
C = 8
NLVL = 10
BF16 = ml_dtypes.bfloat16

_CACHE = {}


# ---------------------------------------------------------------- host prep

def _host_prep(inputs):
    """Build the per-core input maps (numpy only)."""
    ind = np.ascontiguousarray(np.asarray(inputs["in_data"], np.float32))
    f = {l: np.asarray(inputs[f"f{l}"], np.float32) for l in range(1, NLVL + 1)}
    f0 = np.asarray(inputs["in_filter"], np.float32)     # [2, 1, 8]
    fd = np.asarray(inputs["fea_dense"], np.float32)     # [1024, 8, 2]

    shared = {}
    # r0 [32, 64, 16]: r0[row, wHi, b] = in[b, wHi*32 + row]
    shared["r0"] = np.ascontiguousarray(
        ind[:, :, 0].reshape(B, 64, 32).transpose(2, 1, 0))

    # w0 [32, 128]: rows (2*wsub + k), cols (wsub*8 + co)
    w0 = np.zeros((32, 128), np.float32)
    for wsub in range(16):
        for k in range(2):
            w0[2 * wsub + k, wsub * 8:wsub * 8 + 8] = f0[k, 0, :]
    shared["w0"] = w0

    # packed levels 1..4 stacked: wpk [4, 128, 128]
    wpk = np.zeros((4, 128, 128), np.float32)
    for lvl in range(1, 5):
        cin = 2 ** (lvl - 1) * C
        cout = 2 ** lvl * C
        s_out = (128 // cin) // 2
        for wso in range(s_out):
            for k in range(2):
                wsi = 2 * wso + k
                wpk[lvl - 1, wsi * cin:(wsi + 1) * cin,
                    wso * cout:(wso + 1) * cout] = f[lvl][k]
    shared["wpk"] = wpk

    # w5/w6/w7 mega-packed [128, 10752] bf16 (kt-major per level), one DMA
    w5v = f[5].astype(BF16).reshape(2, 1, 128, 256)
    w6v = f[6].astype(BF16).reshape(2, 2, 128, 512)
    w7v = f[7].astype(BF16).reshape(2, 4, 128, 1024)
    shared["wmid"] = np.ascontiguousarray(np.concatenate([
        w5v.transpose(2, 0, 1, 3).reshape(128, 512),
        w6v.transpose(2, 0, 1, 3).reshape(128, 2048),
        w7v.transpose(2, 0, 1, 3).reshape(128, 8192)], axis=1))

    # f8/f9/f10 output-channel shards, packed into 4-ci-tile chunks:
    # [nchunks, 128, 4, cout_shard]; chunk m = k*(cin//512) + q, cit = q*4+j
    def shard_pack(fl, cin):
        sh = fl.shape[-1] // NCORES
        out = []
        flb = fl.astype(BF16)
        for r in range(NCORES):
            blk = flb[:, :, r * sh:(r + 1) * sh]
            v = blk.reshape(2, cin // 512, 4, 128, sh).transpose(0, 1, 3, 2, 4)
            out.append(np.ascontiguousarray(
                v.reshape(2 * (cin // 512), 128, 4, sh)))
        return out

    # f8 is REPLICATED (cheaper than the extra AllGather): co-major chunks
    # [4, 128, kt=16, co=512], kt = k*8 + cit
    f8b = f[8].astype(BF16)
    w8full = np.stack([
        np.ascontiguousarray(
            f8b[:, :, c * 512:(c + 1) * 512]
            .reshape(2, 8, 128, 512).transpose(2, 0, 1, 3).reshape(128, 16, 512))
        for c in range(4)])
    shared["w8"] = w8full

    w9s = shard_pack(f[9], 2048)    # 8 chunks of [128, 4, 512]
    w10s = shard_pack(f[10], 4096)  # 16 chunks of [128, 4, 1024]

    # fea_dense shard, per-o flattened, tiled over the 16 batch partitions
    fds = []
    for r in range(NCORES):
        blk = fd[r * 128:(r + 1) * 128]                    # [128, 8, 2]
        flat = blk.transpose(2, 0, 1).reshape(2, 1024)     # [o, 1024]
        fds.append(np.ascontiguousarray(
            np.broadcast_to(flat[None], (B, 2, 1024))))

    in_maps = []
    for r in range(NCORES):
        m = dict(shared)
        m["w9"] = w9s[r]
        m["w10"] = w10s[r]
        m["fdt"] = fds[r]
        in_maps.append(m)
    return in_maps


# ---------------------------------------------------------------- bass build

def _build():
    import concourse.bass as bass
    import concourse.mybir as mybir
    import concourse.tile as tile
    from concourse import bacc

    f32 = mybir.dt.float32
    bf16 = mybir.dt.bfloat16
    RELU = mybir.ActivationFunctionType.Relu

    nc = bacc.Bacc("TRN2", target_bir_lowering=False, debug=False,
                   num_devices=NCORES)

    def inp(name, shape, dt=f32):
        return nc.dram_tensor(name, shape, dt, kind="ExternalInput").ap()

    r0 = inp("r0", [32, 64, 16])
    w0 = inp("w0", [32, 128])
    wpk = inp("wpk", [4, 128, 128])
    wmid = inp("wmid", [128, 10752], bf16)
    w8 = inp("w8", [4, 128, 16, 512], bf16)
    w9 = inp("w9", [8, 128, 4, 512], bf16)
    w10 = inp("w10", [16, 128, 4, 1024], bf16)
    fdt = inp("fdt", [B, 2, 1024])
    out = nc.dram_tensor("out", [B, 128, 2], f32, kind="ExternalOutput").ap()

    with tile.TileContext(nc) as tc:
        with (
            tc.tile_pool(name="const", bufs=1) as constp,
            tc.tile_pool(name="actp", bufs=3) as actp,
            tc.tile_pool(name="bigp", bufs=1) as bigp,
            tc.tile_pool(name="w7p", bufs=1) as w7p,
            tc.tile_pool(name="w8p", bufs=2) as w8p,
            tc.tile_pool(name="w9p", bufs=6) as w9p,
            tc.tile_pool(name="w10p", bufs=7) as w10p,
            tc.tile_pool(name="psA", bufs=2, space="PSUM") as psA,
            tc.tile_pool(name="psB", bufs=4, space="PSUM") as psB,
            tc.tile_pool(name="psC", bufs=2, space="PSUM") as psC,
            tc.tile_pool(name="dramp", bufs=1, space="DRAM") as dramp,
        ):
            # ---- resident loads
            r0sb = constp.tile([32, 64, 16], f32, name="r0sb")
            nc.sync.dma_start(r0sb[:], r0)
            w0sb = constp.tile([32, 128], f32, name="w0sb")
            nc.sync.dma_start(w0sb[:], w0)
            wpksb = constp.tile([128, 4, 128], f32, name="wpksb")
            nc.sync.dma_start(wpksb[:], wpk.rearrange("l p c -> p l c"))
            wmidsb = w7p.tile([128, 10752], bf16, name="wmidsb")
            nc.sync.dma_start(wmidsb[:], wmid)
            w5sb = wmidsb[:, 0:512].rearrange("p (t c) -> p t c", c=256)
            w6sb = wmidsb[:, 512:2560].rearrange("p (t c) -> p t c", c=512)
            w7sb = wmidsb[:, 2560:10752].rearrange("p (t c) -> p t c", c=1024)
            fdsb = constp.tile([B, 2, 1024], f32, name="fdsb")
            nc.sync.dma_start(fdsb[:], fdt)

            # ---- input conv + packed levels 1..4 (all [128, 64, 16])
            xprev = None
            for lvl in range(5):
                # x4 feeds the bf16 level-5 matmul, so cast at the relu
                xn = actp.tile([128, 64, 16], bf16 if lvl == 4 else f32,
                               name=f"x{lvl}", tag="xl")
                for ch in range(2):
                    ps = psA.tile([128, 32, 16], f32, name="psA", tag="psA")
                    if lvl == 0:
                        nc.tensor.matmul(
                            ps[:], w0sb[:], r0sb[:, ch * 32:(ch + 1) * 32, :],
                            start=True, stop=True)
                    else:
                        nc.tensor.matmul(
                            ps[:], wpksb[:, lvl - 1, :],
                            xprev[:, ch * 32:(ch + 1) * 32, :],
                            start=True, stop=True)
                    nc.scalar.activation(
                        xn[:, ch * 32:(ch + 1) * 32, :], ps[:], RELU)
                xprev = xn

            # ---- standard levels (orientation A, weights stationary)
            def std_level(xin, wsb, cin_t, cout_t, w_out, name, out_tile=None):
                # xin [128, cin_t, 2*w_out, 16]; wsb [128, 2*cin_t, co] with
                # kt = k*cin_t + cit; returns [128, cout_t, w_out, 16]
                if out_tile is None:
                    xn = actp.tile([128, cout_t, w_out, 16], bf16,
                                   name=name, tag="xl")
                else:
                    xn = out_tile
                for ct in range(cout_t):
                    ps = psA.tile([128, w_out, 16], f32, name="psA", tag="psA")
                    for cit in range(cin_t):
                        rhs2 = xin[:, cit].rearrange(
                            "p (w two) b -> p two w b", two=2)
                        for k in range(2):
                            nc.tensor.matmul(
                                ps[:],
                                wsb[:, k * cin_t + cit,
                                    ct * 128:(ct + 1) * 128],
                                rhs2[:, k],
                                start=(cit == 0 and k == 0),
                                stop=(cit == cin_t - 1 and k == 1))
                    nc.scalar.activation(xn[:, ct], ps[:], RELU)
                return xn

            x5 = std_level(xprev[:, None], w5sb, 1, 2, 32, "x5")
            x6 = std_level(x5, w6sb, 2, 4, 16, "x6")
            x7 = std_level(x6, w7sb, 4, 8, 8, "x7")

            # ---- level 8 REPLICATED (full 2048 cout), co-major weight stream
            x8sb = bigp.tile([128, 16, 4, 16], bf16, name="x8sb")
            for c in range(4):
                w8c = w8p.tile([128, 16, 512], bf16, name="w8c", tag="w8c")
                nc.sync.dma_start(w8c[:], w8[c])
                for ctl in range(4):
                    ps = psA.tile([128, 4, 16], f32, name="psA", tag="psA")
                    for cit in range(8):
                        rhs2 = x7[:, cit].rearrange(
                            "p (w two) b -> p two w b", two=2)
                        for k in range(2):
                            nc.tensor.matmul(
                                ps[:],
                                w8c[:, k * 8 + cit, ctl * 128:(ctl + 1) * 128],
                                rhs2[:, k],
                                start=(cit == 0 and k == 0),
                                stop=(cit == 7 and k == 1))
                    nc.scalar.activation(x8sb[:, c * 4 + ctl], ps[:], RELU)

            # ---- level 9 (512-ch shard, streamed weights, 4 accumulators)
            ps9 = [psB.tile([128, 2, 16], f32, name=f"ps9_{ct}", tag="psB")
                   for ct in range(4)]
            for m in range(8):
                w9c = w9p.tile([128, 4, 512], bf16, name="w9c", tag="w9c")
                nc.sync.dma_start(w9c[:], w9[m])
                k, q = divmod(m, 4)
                for j in range(4):
                    cit = q * 4 + j
                    rhs = x8sb[:, cit].rearrange(
                        "p (w two) b -> p two w b", two=2)[:, k]
                    for ct in range(4):
                        nc.tensor.matmul(
                            ps9[ct][:],
                            w9c[:, j, ct * 128:(ct + 1) * 128],
                            rhs,
                            start=(m == 0 and j == 0),
                            stop=(m == 7 and j == 3))
            x9loc = bigp.tile([128, 4, 2, 16], bf16, name="x9loc")
            for ct in range(4):
                nc.scalar.activation(x9loc[:, ct], ps9[ct][:], RELU)

            # ---- AllGather x9 -> full [128, 32, 2, 16]
            ag9_in = dramp.tile([1, 128, 4, 2, 16], bf16, name="ag9_in")
            ag9_out = dramp.tile([NCORES, 128, 4, 2, 16], bf16, name="ag9_out",
                                 addr_space="Shared")
            nc.sync.dma_start(ag9_in[0], x9loc[:])
            nc.gpsimd.collective_compute(
                "AllGather", mybir.AluOpType.bypass,
                replica_groups=[list(range(NCORES))],
                ins=[ag9_in.opt()], outs=[ag9_out.opt()])
            x9sb = bigp.tile([128, 32, 2, 16], bf16, name="x9sb")
            for r in range(NCORES):
                nc.sync.dma_start(x9sb[:, 4 * r:4 * r + 4], ag9_out[r])

            # ---- level 10 (1024-ch shard, orientation B: acts stationary)
            ps10 = [psC.tile([B, 512], f32, name=f"ps10_{cb}", tag="psC")
                    for cb in range(2)]
            for m in range(16):
                w10c = w10p.tile([128, 4, 1024], bf16, name="w10c", tag="w10c")
                nc.sync.dma_start(w10c[:], w10[m])
                k, q = divmod(m, 8)
                for j in range(4):
                    t = q * 4 + j
                    lhsT = x9sb[:, t, k, :]
                    for cb in range(2):
                        nc.tensor.matmul(
                            ps10[cb][:], lhsT,
                            w10c[:, j, cb * 512:(cb + 1) * 512],
                            start=(m == 0 and j == 0),
                            stop=(m == 15 and j == 3))
            x10 = bigp.tile([B, 1024], f32, name="x10")
            for cb in range(2):
                nc.scalar.activation(
                    x10[:, cb * 512:(cb + 1) * 512], ps10[cb][:], RELU)

            # ---- final per-block einsum on the vector engine
            osb = bigp.tile([B, 128, 2], f32, name="osb")
            for o in range(2):
                prod = bigp.tile([B, 1024], f32, name=f"prod{o}")
                nc.vector.tensor_tensor(
                    prod[:], x10[:], fdsb[:, o, :], mybir.AluOpType.mult)
                nc.vector.tensor_reduce(
                    osb[:, :, o],
                    prod.rearrange("p (k c) -> p k c", c=8),
                    mybir.AxisListType.X, mybir.AluOpType.add)
            nc.sync.dma_start(out, osb[:])

    nc.compile()
    return nc


# ------------------------------------------------------------------- kernel

def kernel(**inputs):
    from concourse.bass_utils import run_bass_kernel_spmd

    in_maps = _host_prep(inputs)
    if "nc" not in _CACHE:
        _CACHE["nc"] = _build()
    nc = _CACHE["nc"]
    res = run_bass_kernel_spmd(nc, in_maps, core_ids=list(range(NCORES)))
    parts = [res.results[r]["out"] for r in range(NCORES)]  # each [16, 128, 2]
    full = np.concatenate(parts, axis=1)                    # [16, 1024, 2]
    return np.ascontiguousarray(full.reshape(B, 2048, 1).astype(np.float32))
